# revision 1
# baseline (speedup 1.0000x reference)
"""Trainium2 Bass kernel for nn_DiscoveryMemorywithDynamicThreshold.

Reference computation (per batch of 32 samples):
  1. 1x1 conv projection 512->256 channels (+bias)          proj = W @ feats + b
  2. preds-masked average pool over HW                       pooled[b] = mean_l(proj*preds)
  3. sequential memory-bank update over the 32 samples       (cos-sim match -> EMA or append)
  4. cross-attention of proj against the memory bank         aug = mem^T softmax(mem @ proj)
  5. output = concat([proj, aug], channel axis)

Sharding: data-parallel over batch (4 batches per core x 8 cores).  The
serial memory scan operates only on the 32x32 Gram matrix of the pooled
vectors, so each core AllGathers the tiny pooled shards [4,256] and runs the
scan redundantly (branchless dataflow formulation; memory columns are linear
combinations of pooled vectors, tracked via a 32x32 coefficient matrix).

The heavy matmuls use the PE float32r path (fp22 multiply, fp32 accumulate)
which streams at 1 row/cycle; everything decision-critical (Gram matrix,
scan state, norm bookkeeping) stays true fp32.
"""

import sys

if "/opt/trn_rl_repo" not in sys.path:
    sys.path.insert(0, "/opt/trn_rl_repo")

import numpy as np

import concourse.bacc as bacc
import concourse.bass as bass
import concourse.tile as tile
from concourse.tile import add_dep_helper
from concourse import mybir
from concourse.bass_utils import run_bass_kernel_spmd

F32 = mybir.dt.float32
F32R = mybir.dt.float32r
OP = mybir.AluOpType
ACT = mybir.ActivationFunctionType
X = mybir.AxisListType.X

N_CORES = 8
B_FULL = 32
B_SH = B_FULL // N_CORES          # 4 batches per core
C_IN = 512
C_OUT = 256
HW = 4096
S = 32                            # memory slots actually reachable (<= batch)
L_TILE = 512
N_LT = HW // L_TILE               # 8 l-tiles per batch
BIG = 1.0e30
DECAY = 0.9


def _r(ap):
    return ap.bitcast(F32R)


def _build(threshold: float):
    nc = bacc.Bacc("TRN2", target_bir_lowering=False, debug=False,
                   num_devices=N_CORES)

    feats_t = nc.dram_tensor("feats", [B_SH, C_IN, HW], F32R, kind="ExternalInput")
    preds_t = nc.dram_tensor("preds", [B_SH, HW], F32R, kind="ExternalInput")
    w_t = nc.dram_tensor("w", [C_OUT, C_IN], F32, kind="ExternalInput")
    b_t = nc.dram_tensor("b", [C_OUT], F32, kind="ExternalInput")
    ident_t = nc.dram_tensor("ident", [128, 128], F32, kind="ExternalInput")
    cmask_t = nc.dram_tensor("cmask", [S, 32 * N_LT], F32R, kind="ExternalInput")
    bmask_t = nc.dram_tensor("bmask", [S, 32 * N_LT], F32R, kind="ExternalInput")
    out_t = nc.dram_tensor("out", [B_SH, 2 * C_OUT, HW], F32, kind="ExternalOutput")

    thr2 = float(threshold) * float(threshold)

    with tile.TileContext(nc) as tc:
        with (
            tc.tile_pool(name="persist", bufs=1) as persist,
            tc.tile_pool(name="state", bufs=1) as state,
        ):
            # ---------- persistent SBUF ----------
            id_sb = persist.tile([128, 128], F32)
            nc.sync.dma_start(id_sb[:], ident_t[:])
            i32 = id_sb[:32, :32]

            cmask_sb = persist.tile([S, 32 * N_LT], F32R)
            nc.sync.dma_start(cmask_sb[:], cmask_t[:])
            bmask_sb = persist.tile([S, 32 * N_LT], F32R)
            nc.sync.dma_start(bmask_sb[:], bmask_t[:])

            ones_row = persist.tile([1, 128], F32)
            nc.vector.memset(ones_row[:], 1.0)
            ones_row_r = persist.tile([1, 128], F32R)
            nc.vector.tensor_copy(ones_row_r[:], ones_row[:])
            one1 = persist.tile([1, 1], F32)
            nc.vector.memset(one1[:], 1.0)
            mone1 = persist.tile([1, 1], F32)
            nc.vector.memset(mone1[:], -1.0)

            bcol = persist.tile([128, 2], F32)
            for oh in range(2):
                nc.sync.dma_start(
                    bcol[:, oh:oh + 1],
                    b_t[oh * 128:(oh + 1) * 128].rearrange("(p o) -> p o", o=1),
                )

            # W^T (conv lhsT), via PE transpose of W chunks
            wt_sb = persist.tile([128, 4 * C_OUT], F32R)   # 4 c-chunks x [128, 256]
            proj_sb0 = persist.tile([128, B_SH * HW], F32R)
            proj_sb1 = persist.tile([128, B_SH * HW], F32R)
            proj_sb = [proj_sb0, proj_sb1]

            # scan / pooled state
            pooled_sb = state.tile([128, 2 * B_SH], F32)      # [c-half, 2*b]
            pooledT_sb = state.tile([B_SH, C_OUT], F32)
            pag_sb = state.tile([B_FULL, C_OUT], F32)         # allgathered pooled
            pcb_sb = state.tile([128, 2 * B_FULL], F32)       # [c, b] both halves
            d0_sb = state.tile([S, S], F32)
            dcol = state.tile([S, 1], F32)
            xsq_sb = state.tile([1, S], F32)
            th2_sb = state.tile([1, S], F32)
            dots_m = state.tile([S, S], F32)                  # DOTS[j, s]
            coef = state.tile([S, S], F32)                    # coef[s, b]
            n2 = state.tile([1, S], F32)
            p1h = state.tile([1, S], F32)
            sh = state.tile([1, S], F32)
            coefT_sb = state.tile([S, S], F32)
            memT_sb = state.tile([128, 2 * S], F32R)           # [c, s] both halves
            mem_sb = state.tile([S, C_OUT], F32R)              # [s, c]
            penc_sb = state.tile([S, 1], F32)

            with tc.tile_pool(name="prep_ps", bufs=2, space="PSUM") as prep_ps:
                with tc.tile_pool(name="wtmp", bufs=1) as wtmp:
                    w_sb = wtmp.tile([128, 2 * C_IN], F32)
                    for oh in range(2):
                        nc.sync.dma_start(
                            w_sb[:, oh * C_IN:(oh + 1) * C_IN],
                            w_t[oh * 128:(oh + 1) * 128, :],
                        )
                    for oh in range(2):
                        for kc in range(4):
                            tp = prep_ps.tile([128, 128], F32, tag="prep")
                            nc.tensor.transpose(
                                tp[:],
                                w_sb[:, oh * C_IN + kc * 128: oh * C_IN + (kc + 1) * 128],
                                id_sb[:],
                            )
                            nc.vector.tensor_copy(
                                wt_sb[:, kc * C_OUT + oh * 128: kc * C_OUT + (oh + 1) * 128],
                                tp[:],
                            )

                # ---------- phase 1: conv + masked pooling ----------
                with (
                    tc.tile_pool(name="fpool", bufs=2) as fpool,
                    tc.tile_pool(name="prpool", bufs=3) as prpool,
                    tc.tile_pool(name="scrpool", bufs=2) as scrpool,
                    tc.tile_pool(name="pcpool", bufs=2) as pcpool,
                    tc.tile_pool(name="conv_ps", bufs=3, space="PSUM") as conv_ps,
                    tc.tile_pool(name="pbc_ps", bufs=2, space="PSUM") as pbc_ps,
                ):
                    for b in range(B_SH):
                        pc0 = pcpool.tile([128, N_LT], F32, tag="pc0")
                        pc1 = pcpool.tile([128, N_LT], F32, tag="pc1")
                        pcs = [pc0, pc1]
                        for lq in range(4):
                            fch = []
                            for kc in range(4):
                                f = fpool.tile([128, 1024], F32R, tag=f"f{kc}")
                                nc.sync.dma_start(
                                    f[:],
                                    feats_t[b, kc * 128:(kc + 1) * 128,
                                            lq * 1024:(lq + 1) * 1024],
                                )
                                fch.append(f)
                            prow = prpool.tile([1, 1024], F32R, tag="prow")
                            nc.sync.dma_start(
                                prow[:], preds_t[b:b + 1, lq * 1024:(lq + 1) * 1024]
                            )
                            for lt2 in range(2):
                                lt = lq * 2 + lt2
                                col = b * N_LT + lt
                                pbc = pbc_ps.tile([128, L_TILE], F32, tag="pbc")
                                nc.tensor.matmul(
                                    pbc[:], ones_row_r[:, :128],
                                    prow[:, lt2 * 512:(lt2 + 1) * 512],
                                    start=True, stop=True,
                                )
                                for oh in range(2):
                                    ps = conv_ps.tile([128, L_TILE], F32, tag="cv")
                                    for kc in range(4):
                                        nc.tensor.matmul(
                                            ps[:],
                                            wt_sb[:, kc * C_OUT + oh * 128:
                                                     kc * C_OUT + (oh + 1) * 128],
                                            fch[kc][:, lt2 * 512:(lt2 + 1) * 512],
                                            start=(kc == 0), stop=(kc == 3),
                                        )
                                    pslice = proj_sb[oh][:, col * 512:(col + 1) * 512]
                                    nc.scalar.activation(
                                        pslice, ps[:], ACT.Identity,
                                        bias=bcol[:, oh:oh + 1], scale=1.0,
                                    )
                                    scr = scrpool.tile([128, L_TILE], F32, tag="scr")
                                    nc.vector.scalar_tensor_tensor(
                                        scr[:], pslice, 1.0 / HW, pbc[:],
                                        OP.mult, OP.mult,
                                        accum_out=pcs[oh][:, lt:lt + 1],
                                    )
                        for oh in range(2):
                            nc.vector.reduce_sum(
                                pooled_sb[:, oh * B_SH + b: oh * B_SH + b + 1],
                                pcs[oh][:], X,
                            )

                # ---------- phase 1b: allgather pooled, Gram matrix ----------
                for oh in range(2):
                    tp = prep_ps.tile([B_SH, 128], F32, tag="prep")
                    nc.tensor.transpose(
                        tp[:], pooled_sb[:, oh * B_SH:(oh + 1) * B_SH], id_sb[:]
                    )
                    nc.vector.tensor_copy(
                        pooledT_sb[:, oh * 128:(oh + 1) * 128], tp[:]
                    )

                with tc.tile_pool(name="dram", bufs=1, space="DRAM") as dram:
                    agin = dram.tile([B_SH, C_OUT], F32)
                    agout = dram.tile([B_FULL, C_OUT], F32)
                    nc.gpsimd.dma_start(agin[:], pooledT_sb[:])
                    nc.gpsimd.collective_compute(
                        "AllGather", OP.bypass,
                        replica_groups=[list(range(N_CORES))],
                        ins=[agin.opt()], outs=[agout.opt()],
                    )
                    nc.gpsimd.dma_start(pag_sb[:], agout[:])

                # proj DMA-out deferred here (hard-ordered after the
                # collective trigger) so the writes fill the collective +
                # scan window instead of competing with the feats reads
                for b in range(B_SH):
                    for oh in range(2):
                        nc.sync.dma_start(
                            out_t[b, oh * 128:(oh + 1) * 128, :],
                            proj_sb[oh][:, b * HW:(b + 1) * HW].bitcast(F32),
                        )

                for oh in range(2):
                    tp2 = prep_ps.tile([128, B_FULL], F32, tag="prep")
                    nc.tensor.transpose(
                        tp2[:], pag_sb[:, oh * 128:(oh + 1) * 128], i32
                    )
                    nc.vector.tensor_copy(
                        pcb_sb[:, oh * B_FULL:(oh + 1) * B_FULL], tp2[:]
                    )

                d0ps = prep_ps.tile([S, S], F32, tag="prep")
                for oh in range(2):
                    pc = pcb_sb[:, oh * B_FULL:(oh + 1) * B_FULL]
                    nc.tensor.matmul(d0ps[:], pc, pc, start=(oh == 0), stop=(oh == 1))
                nc.vector.tensor_copy(d0_sb[:], d0ps[:])

                scr32 = state.tile([S, S], F32)
                nc.vector.scalar_tensor_tensor(
                    scr32[:], d0_sb[:], 1.0, i32, OP.mult, OP.mult,
                    accum_out=dcol[:],
                )
                xsqps = prep_ps.tile([1, S], F32, tag="prep")
                nc.tensor.matmul(xsqps[:], dcol[:], i32, start=True, stop=True)
                nc.vector.tensor_copy(xsq_sb[:], xsqps[:])
                nc.vector.tensor_scalar(th2_sb[:], xsq_sb[:], thr2, None, OP.mult)

                # scan init (step 0 always appends into slot 0)
                nc.vector.memset(dots_m[:], 0.0)
                nc.vector.tensor_copy(dots_m[:, 0:1], d0_sb[:, 0:1])
                nc.vector.memset(n2[:], BIG)
                nc.vector.tensor_copy(n2[:, 0:1], xsq_sb[:, 0:1])
                nc.vector.memset(p1h[:], 0.0)
                nc.vector.memset(p1h[:, 1:2], 1.0)
                nc.vector.memset(sh[:], 0.0)
                nc.vector.memset(coef[:], 0.0)
                nc.vector.memset(coef[0:1, 0:1], 1.0)

            # ---------- phase 2: serial scan over samples 1..31 ----------
            with (
                tc.tile_pool(name="rows", bufs=3) as rows,
                tc.tile_pool(name="m32", bufs=3) as m32,
                tc.tile_pool(name="dots_ps", bufs=2, space="PSUM") as dots_psp,
                tc.tile_pool(name="bca_ps", bufs=2, space="PSUM") as bca_psp,
                tc.tile_pool(name="awc_ps", bufs=2, space="PSUM") as awc_psp,
            ):
                for i in range(1, B_FULL):
                    rcp = rows.tile([1, S], F32, tag="rcp")
                    nc.vector.reciprocal(rcp[:], n2[:])
                    dots = dots_psp.tile([1, S], F32, tag="dots")
                    nc.tensor.matmul(dots[:], id_sb[:32, i:i + 1], dots_m[:],
                                     start=True, stop=True)
                    r = rows.tile([1, S], F32, tag="r")
                    nc.scalar.activation(r[:], dots[:], ACT.Relu)
                    rq = rows.tile([1, S], F32, tag="rq")
                    nc.scalar.activation(rq[:], r[:], ACT.Square)
                    sims = rows.tile([1, S], F32, tag="sims")
                    nc.vector.tensor_mul(sims[:], rq[:], rcp[:])
                    rmax = rows.tile([1, 1], F32, tag="rmax")
                    nc.vector.reduce_max(rmax[:], sims[:], X)
                    weq = rows.tile([1, S], F32, tag="weq")
                    nc.vector.tensor_scalar(weq[:], sims[:], rmax[0:1, 0:1], None,
                                            OP.is_ge)
                    d = rows.tile([1, 1], F32, tag="d")
                    nc.vector.tensor_tensor(d[:], rmax[:], th2_sb[0:1, i:i + 1],
                                            OP.is_ge)
                    u1 = rows.tile([1, S], F32, tag="u1")
                    nc.vector.tensor_sub(u1[:], weq[:], p1h[:])
                    w = rows.tile([1, S], F32, tag="w")
                    nc.vector.scalar_tensor_tensor(w[:], u1[:], d[0:1, 0:1],
                                                   p1h[:], OP.mult, OP.add)
                    v1 = rows.tile([1, S], F32, tag="v1")
                    nc.vector.scalar_tensor_tensor(v1[:], weq[:], 1.0 - DECAY,
                                                   p1h[:], OP.mult, OP.subtract)
                    aw = rows.tile([1, S], F32, tag="aw")
                    nc.vector.scalar_tensor_tensor(aw[:], v1[:], d[0:1, 0:1],
                                                   p1h[:], OP.mult, OP.add)
                    bca = bca_psp.tile([S, S], F32, tag="bca")
                    nc.tensor.matmul(bca[:], ones_row[:, :S], aw[:],
                                     start=True, stop=True)
                    # awcn = -aw as a column (negated so the coef update fuses)
                    awcn = awc_psp.tile([S, 1], F32, tag="awc")
                    nc.tensor.matmul(awcn[:], aw[:], mone1[:], start=True, stop=True)
                    dm = m32.tile([S, S], F32, tag="dm")
                    nc.vector.scalar_tensor_tensor(dm[:], dots_m[:], d0_sb[:, i:i + 1],
                                                   bca[:], OP.subtract, OP.mult)
                    nc.vector.tensor_sub(dots_m[:], dots_m[:], dm[:])
                    # coefficient matrix update: coef = coef*(-aw) + coef, col i -= -aw
                    nc.vector.scalar_tensor_tensor(coef[:], coef[:], awcn[:, 0:1],
                                                   coef[:], OP.mult, OP.add)
                    nc.vector.tensor_sub(coef[:, i:i + 1], coef[:, i:i + 1], awcn[:])
                    # squared-norm bookkeeping, multiplicatively masked so the
                    # 1e30 "unused" sentinel cancels exactly on append:
                    #   z = d*D^2*n2 + d*2D(1-D)*dots + (1-(1-(1-D)^2)*d)*xsq
                    #   n2 = n2*(1-w) + w*z
                    a2 = rows.tile([1, 1], F32, tag="a2")
                    nc.scalar.activation(a2[:], d[:], ACT.Copy, scale=DECAY * DECAY)
                    s18 = rows.tile([1, 1], F32, tag="s18")
                    nc.scalar.activation(s18[:], d[:], ACT.Copy,
                                         scale=2.0 * DECAY * (1.0 - DECAY))
                    ds18 = rows.tile([1, S], F32, tag="ds18")
                    nc.scalar.activation(ds18[:], dots[:], ACT.Copy,
                                         scale=s18[0:1, 0:1])
                    zz = rows.tile([1, S], F32, tag="zz")
                    nc.vector.scalar_tensor_tensor(zz[:], n2[:], a2[0:1, 0:1],
                                                   ds18[:], OP.mult, OP.add)
                    dxs = rows.tile([1, 1], F32, tag="dxs")
                    nc.scalar.activation(dxs[:], xsq_sb[0:1, i:i + 1], ACT.Copy,
                                         scale=d[0:1, 0:1])
                    qx = rows.tile([1, 1], F32, tag="qx")
                    nc.scalar.activation(
                        qx[:], dxs[:], ACT.Identity,
                        scale=(1.0 - DECAY) * (1.0 - DECAY) - 1.0,
                        bias=xsq_sb[0:1, i:i + 1])
                    z2 = rows.tile([1, S], F32, tag="z2")
                    nc.scalar.activation(z2[:], zz[:], ACT.Identity,
                                         bias=qx[0:1, 0:1])
                    wz = rows.tile([1, S], F32, tag="wz")
                    nc.vector.tensor_mul(wz[:], z2[:], w[:])
                    nw = rows.tile([1, S], F32, tag="nw")
                    nc.vector.tensor_mul(nw[:], n2[:], w[:])
                    nc.vector.tensor_sub(n2[:], n2[:], nw[:])
                    nc.vector.tensor_add(n2[:], n2[:], wz[:])
                    # pointer one-hot shift
                    nc.scalar.copy(sh[0:1, 1:S], p1h[0:1, 0:S - 1])
                    u2 = rows.tile([1, S], F32, tag="u2")
                    nc.vector.tensor_sub(u2[:], p1h[:], sh[:])
                    nc.vector.scalar_tensor_tensor(p1h[:], u2[:], d[0:1, 0:1],
                                                   sh[:], OP.mult, OP.add)

            # ---------- phase 3: memory build + cross-attention ----------
            with (
                tc.tile_pool(name="att_sb", bufs=1) as att_sb,
                tc.tile_pool(name="epool", bufs=2) as epool,
                tc.tile_pool(name="opool", bufs=6) as opool,
                tc.tile_pool(name="att_ps", bufs=1, space="PSUM") as att_ps,
            ):
                ctps = att_ps.tile([S, S], F32, tag="aug", bufs=2)
                nc.tensor.transpose(ctps[:], coef[:], i32)
                nc.vector.tensor_copy(coefT_sb[:], ctps[:])
                for oh in range(2):
                    mps = att_ps.tile([128, S], F32, tag="aug", bufs=2)
                    nc.tensor.matmul(mps[:], pag_sb[:, oh * 128:(oh + 1) * 128],
                                     coefT_sb[:], start=True, stop=True)
                    nc.vector.tensor_copy(memT_sb[:, oh * S:(oh + 1) * S], mps[:])
                msps = att_ps.tile([S, C_OUT], F32, tag="aug", bufs=2)
                nc.tensor.matmul(msps[:], coefT_sb[:], pag_sb[:], start=True, stop=True)
                nc.vector.tensor_copy(mem_sb[:], msps[:])

                val = att_sb.tile([1, S], F32)
                nc.vector.tensor_scalar(val[:], n2[:], 0.1 * BIG, None, OP.is_lt)
                pen = att_sb.tile([1, S], F32)
                nc.vector.tensor_scalar(pen[:], val[:], 1.0, BIG, OP.subtract, OP.mult)
                pps = att_ps.tile([S, 1], F32, tag="lg", bufs=2)
                nc.tensor.matmul(pps[:], pen[:], one1[:], start=True, stop=True)
                nc.vector.tensor_copy(penc_sb[:], pps[:])

                zrow = att_sb.tile([S, L_TILE], F32)
                nc.vector.memset(zrow[:], 0.0)

                for b in range(B_SH):
                    e_all = epool.tile([S, HW], F32R, tag="e_all")
                    den_sb = att_sb.tile([S, L_TILE], F32, tag="den_sb", bufs=2)
                    rcpd = att_sb.tile([S, L_TILE], F32R, tag="rcpd", bufs=2)
                    nc.vector.tensor_copy(rcpd[:], zrow[:])
                    den = att_ps.tile([S, L_TILE], F32, tag="den", bufs=2)
                    for lt in range(N_LT):
                        col = b * N_LT + lt
                        lg = att_ps.tile([S, L_TILE], F32, tag="lg", bufs=2)
                        for oh in range(2):
                            nc.tensor.matmul(
                                lg[:],
                                memT_sb[:, oh * S:(oh + 1) * S],
                                proj_sb[oh][:, col * 512:(col + 1) * 512],
                                start=(oh == 0), stop=(oh == 1),
                            )
                        nc.scalar.activation(e_all[:, lt * 512:(lt + 1) * 512], lg[:],
                                             ACT.Exp, bias=penc_sb[:, 0:1], scale=1.0)
                        nc.tensor.matmul(
                            den[:],
                            cmask_sb[:, lt * 32:(lt + 1) * 32],
                            e_all[:, lt * 512:(lt + 1) * 512],
                            start=(lt == 0), stop=(lt == N_LT - 1),
                        )
                    nc.vector.tensor_copy(den_sb[:], den[:])
                    rcf = att_sb.tile([N_LT, L_TILE], F32, tag="rcf", bufs=2)
                    rcs = att_sb.tile([N_LT, L_TILE], F32, tag="rcs", bufs=2)
                    nc.vector.reciprocal_approx_accurate(rcf[:], den_sb[:N_LT, :],
                                                         rcs[:])
                    nc.vector.tensor_copy(rcpd[:N_LT, :], rcf[:])
                    for lt in range(N_LT):
                        col = b * N_LT + lt
                        rbc = att_ps.tile([S, L_TILE], F32, tag="rbc", bufs=2)
                        nc.tensor.matmul(rbc[:],
                                         bmask_sb[:, lt * 32:(lt + 1) * 32],
                                         rcpd[:], start=True, stop=True)
                        esl = e_all[:, lt * 512:(lt + 1) * 512]
                        nc.vector.tensor_mul(esl, esl, rbc[:])
                        for oh in range(2):
                            aug = att_ps.tile([128, L_TILE], F32, tag="aug", bufs=2)
                            nc.tensor.matmul(
                                aug[:],
                                mem_sb[:, oh * 128:(oh + 1) * 128],
                                esl, start=True, stop=True,
                            )
                            o = opool.tile([128, L_TILE], F32, tag="o")
                            if (2 * lt + oh) % 4 != 3:
                                nc.scalar.copy(o[:], aug[:])
                            else:
                                nc.vector.tensor_copy(o[:], aug[:])
                            nc.sync.dma_start(
                                out_t[b, C_OUT + oh * 128:C_OUT + (oh + 1) * 128,
                                      lt * 512:(lt + 1) * 512],
                                o[:],
                            )

    nc.compile()
    return nc


_CACHE: dict = {}


def _get_program(threshold: float):
    key = round(float(threshold), 9)
    if key not in _CACHE:
        _CACHE[key] = _build(threshold)
    return _CACHE[key]


def kernel(feats, preds, W, b, epoch):
    feats = np.ascontiguousarray(np.asarray(feats, dtype=np.float32))
    preds = np.ascontiguousarray(np.asarray(preds, dtype=np.float32))
    W = np.ascontiguousarray(np.asarray(W, dtype=np.float32))
    b = np.ascontiguousarray(np.asarray(b, dtype=np.float32))
    epoch = int(np.asarray(epoch))

    threshold = (epoch / 10 - 2) * 0.4 / 13 + 0.3
    assert threshold > 0.0, "kernel assumes a positive match threshold"

    B, C, H, Wd = feats.shape
    assert (B, C, H * Wd) == (B_FULL, C_IN, HW)

    nc = _get_program(threshold)

    ident = np.eye(128, dtype=np.float32)
    # cmask[:, 32*t + m] = 1{m == t}; bmask[:, 32*t + m] = 1{s == t}
    cmask = np.zeros((S, 32 * N_LT), dtype=np.float32)
    bmask = np.zeros((S, 32 * N_LT), dtype=np.float32)
    for t in range(N_LT):
        cmask[:, 32 * t + t] = 1.0
        bmask[t, 32 * t:32 * (t + 1)] = 1.0

    feats_r = feats.reshape(B_FULL, C_IN, HW)
    preds_r = preds.reshape(B_FULL, HW)

    in_maps = []
    for r in range(N_CORES):
        lo, hi = r * B_SH, (r + 1) * B_SH
        in_maps.append({
            "feats": feats_r[lo:hi],
            "preds": preds_r[lo:hi],
            "w": W,
            "b": b,
            "ident": ident,
            "cmask": cmask,
            "bmask": bmask,
        })

    res = run_bass_kernel_spmd(nc, in_maps, core_ids=list(range(N_CORES)))
    out = np.concatenate([res.results[r]["out"] for r in range(N_CORES)], axis=0)
    return out.reshape(B_FULL, 2 * C_OUT, H, Wd)

